# revision 22
# baseline (speedup 1.0000x reference)
"""Multi-head cross-attention on 8 Trainium2 NeuronCores.

Problem (hardcoded): input [4, 2048, 1024], memory [4, 2048, 1024],
Wq/Wk/Wv [1024, 1024], bq/bk/bv [1024]; 16 heads x 64 dim; out
[4, 2048, 1024] f32.

Sharding: core c handles batch b = c//2 and head group g = c%2 (8
heads, output columns 512g:512g+512). Embarrassingly parallel - no
collectives.

Device dataflow (per core), everything contracted over channels with
host-pre-transposed operands so no on-chip transposes are needed:
  Q^T[d, n]  = sum_c WqT[c, d] * XT[c, n]     (depth scale folded in WqT)
  K^T[d, m]  = sum_c WkT[c, d] * MT[c, m]
  V[m, d]    = sum_c MT[c, m] * WvT[c, d]
  S^T[m, q]  = sum_d K^T[d, m] * Q^T[d, q]    (per head; heads of a pair
                                               row-packed in the PE array)
  P^T        = exp(S^T)                        (no max subtraction;
                                               logits are O(5), safe)
  O[q, d]    = sum_m P^T[m, q]^T [V | 1][m, d] ("flipped" PV: P^T chunk
               is the STATIONARY operand, [V|1] (65 cols) the moving one;
               65-cycle streams instead of 512 - half the PE cycles of
               the unflipped form. Ones column gives softmax sums. pso
               tiles are exactly one PSUM bank; only the first matmul
               into a tile carries start=True since start zeroes the
               whole bank.)
Host divides O[:, :64]/O[:, 64] (softmax normalization) and interleaves
head columns. Biases (zero here) are handled exactly via an extra K=1
contraction chunk when any bias is nonzero.

Engine budget: PE is the bottleneck (projections 82us + S 109us +
flipped PV 58us of stream + issue overhead). Exp of 33.5M logits costs
~1.15us per [128,1024] tile on ScalarE (292us > PE), so every 3rd tile
runs on DVE via a 1-pass Schraudolph exp in bf16 bit space:
i16 = round(S*128/ln2 + (127*128-7.42)), bitcast bf16 ~ exp(S) (1.8%
RMS on those tiles -> ~8.6e-3 total vs the 2e-2 gate). Input DMAs are
laid out host-side so every transfer is partition-major contiguous
(2-8KB packets; 128-col strided slices packetize at 256B and run at
~30GB/s vs ~190GB/s). Inputs ride sync HWDGE + a small scalar-ring
prefix (before exp #0 exists) + gpsimd SWDGE; outputs ride SWDGE.
"""

import numpy as np
import ml_dtypes

import concourse.bass as bass
import concourse.mybir as mybir
from concourse import bacc, tile
from concourse.bass_utils import run_bass_kernel_spmd

B, N, M, DIM = 4, 2048, 2048, 1024
NUM_HEADS, HEAD_DIM = 16, 64
HG = 8            # heads per core
COLS = HG * HEAD_DIM  # 512 output cols per core
N_CORES = 8
CC = DIM // 128   # 8 contraction chunks of 128
QC = 4            # q chunks of 512
MC = 16           # m (key) tiles of 128

F32 = mybir.dt.float32
BF16 = mybir.dt.bfloat16
I16 = mybir.dt.int16
EXP = mybir.ActivationFunctionType.Exp
CPY = mybir.ActivationFunctionType.Copy

# Schraudolph exp in bf16 bit space (round-to-nearest f32->i16 on DVE)
SCH_A = float(128.0 / np.log(2.0))
SCH_B = float(127 * 128) - 7.42
# units whose exp runs on DVE (every 3rd -> 1/3 of logits)
DVE_EVERY = 3

_NC_CACHE = {}
_RUN_KWARGS = {}   # test harness may inject trace=True etc.
LAST_RESULT = None


def _build(with_bias: bool):
    """Build the per-core SPMD Bass program."""
    cc_n = CC + (1 if with_bias else 0)
    nc = bacc.Bacc(None, target_bir_lowering=False)

    # All DRAM layouts are per-DMA-piece contiguous (partition-major):
    xt_ext = nc.declare_dram_parameter("xt", [QC, 128, cc_n, 512], BF16,
                                       isOutput=False)
    mt_ext = nc.declare_dram_parameter("mt", [QC, 128, cc_n, 512], BF16,
                                       isOutput=False)
    wq_ext = nc.declare_dram_parameter("wq", [QC, 128, cc_n, 128], BF16,
                                       isOutput=False)
    wk_ext = nc.declare_dram_parameter("wk", [QC, 128, cc_n, 128], BF16,
                                       isOutput=False)
    wv_ext = nc.declare_dram_parameter("wv", [128, cc_n, 512], BF16,
                                       isOutput=False)
    # out[pair, qc, h2] = [128 q rows, 4 q-subtiles, 64 dims + sum]
    out_ext = nc.declare_dram_parameter("out", [QC, QC, 2, 128, 4, 65], F32,
                                        isOutput=True)

    ch = [(i, 128) for i in range(CC)]
    if with_bias:
        ch.append((CC, 1))

    with tile.TileContext(nc) as tc:
        with (
            tc.tile_pool(name="acts", bufs=1) as acts,
            tc.tile_pool(name="qkv", bufs=1) as qkv,
            tc.tile_pool(name="pt", bufs=16) as ptp,
            tc.tile_pool(name="osb", bufs=4) as osb,
            tc.tile_pool(name="ps_proj", bufs=2, space="PSUM") as ps_proj,
            tc.tile_pool(name="ps_s", bufs=2, space="PSUM") as ps_sp,
            tc.tile_pool(name="ps_o", bufs=2, space="PSUM") as ps_op,
        ):
            wk_sb = acts.tile([128, QC, cc_n, 128], BF16)
            wq_sb = acts.tile([128, QC, cc_n, 128], BF16)
            xt_sb = acts.tile([128, QC, cc_n, 512], BF16)
            wv_sb = acts.tile([128, cc_n, 512], BF16)
            mt_sb = acts.tile([128, QC, cc_n, 512], BF16)

            # scalar-ring prefix: K0/Q00 stationaries + wv, all done by
            # ~12us (exp #0 can't issue earlier anyway); clear afterwards.
            nc.scalar.dma_start(wk_sb[:, 0], wk_ext[0])
            nc.scalar.dma_start(wq_sb[:, 0], wq_ext[0])
            nc.scalar.dma_start(wv_sb[:], wv_ext[:])
            # sync ring: mt chunks in K0-consumption order, then the bulk
            for c in range(QC):
                nc.sync.dma_start(mt_sb[:, c], mt_ext[c])
            for g in range(1, QC):
                nc.sync.dma_start(wk_sb[:, g], wk_ext[g])
            for g in range(1, QC):
                nc.sync.dma_start(wq_sb[:, g], wq_ext[g])
            nc.sync.dma_start(xt_sb[:, 2], xt_ext[2])
            nc.sync.dma_start(xt_sb[:, 3], xt_ext[3])
            # gpsimd SWDGE (aggregates packets, ~190GB/s): early xt blocks
            nc.gpsimd.dma_start(xt_sb[:, 0], xt_ext[0])
            nc.gpsimd.dma_start(xt_sb[:, 1], xt_ext[1])

            v_sb = qkv.tile([128, MC, HG, 65], BF16)   # V with ones col
            kt_sb = qkv.tile([128, QC, M], BF16)       # 2-head pairs stacked
            qt_sb = qkv.tile([128, QC, N], BF16)

            nc.gpsimd.memset(v_sb[:, :, :, 64:65], 1.0)

            def copy_ps(on_scalar, dst, src_ap):
                # PSUM->SBUF eviction on whichever exp engine is idle
                # this unit (Copy shares ScalarE's table set with Exp)
                if on_scalar:
                    nc.scalar.activation(dst, src_ap, CPY)
                else:
                    nc.vector.tensor_copy(dst, src_ap)

            def proj_k(pair, mc, on_dve=False):
                ps = ps_proj.tile([128, 512], F32, tag="proj")
                for j, (ci, rows) in enumerate(ch):
                    nc.tensor.matmul(
                        ps[:],
                        wk_sb[:rows, pair, ci, :],
                        mt_sb[:rows, mc, ci, :],
                        start=(j == 0), stop=(j == len(ch) - 1),
                    )
                copy_ps(on_dve, kt_sb[:, pair, mc * 512:(mc + 1) * 512], ps[:])

            def proj_q(pair, qc, on_dve=False):
                ps = ps_proj.tile([128, 512], F32, tag="proj")
                for j, (ci, rows) in enumerate(ch):
                    nc.tensor.matmul(
                        ps[:],
                        wq_sb[:rows, pair, ci, :],
                        xt_sb[:rows, qc, ci, :],
                        start=(j == 0), stop=(j == len(ch) - 1),
                    )
                copy_ps(on_dve, qt_sb[:, pair, qc * 512:(qc + 1) * 512], ps[:])

            def proj_v(mt, on_dve=False):
                ps = ps_proj.tile([128, 512], F32, tag="proj")
                for j, (ci, rows) in enumerate(ch):
                    nc.tensor.matmul(
                        ps[:],
                        mt_sb[:rows, mt // 4, ci,
                              (mt % 4) * 128:(mt % 4 + 1) * 128],
                        wv_sb[:rows, ci, :],
                        start=(j == 0), stop=(j == len(ch) - 1),
                    )
                copy_ps(on_dve, v_sb[:, mt, :, 0:64],
                        ps[:].rearrange("p (h d) -> p h d", h=HG))

            def s_exp(pair, qc, mt, extra, on_dve, split):
                """One m-tile: both heads' S matmuls into one PSUM tile,
                then one exp (ScalarE table exp, or DVE Schraudolph into
                int16-as-bf16 bit space). In split mode both engines
                each take one head's half concurrently - half the exp
                latency for the endgame where no proj padding remains."""
                ps = ps_sp.tile([128, 1024], F32, tag="s")
                for h2 in range(2):
                    d0 = 64 * h2
                    nc.tensor.matmul(
                        ps[:, h2 * 512:(h2 + 1) * 512],
                        kt_sb[d0:d0 + 64, pair, mt * 128:(mt + 1) * 128],
                        qt_sb[d0:d0 + 64, pair, qc * 512:(qc + 1) * 512],
                        start=True, stop=True,
                    )
                for fn in (extra or ()):
                    fn(on_dve)
                if split:
                    pt_i = ptp.tile([128, 1024], I16, tag="pt")
                    nc.scalar.activation(
                        pt_i.bitcast(BF16)[:, 0:512], ps[:, 0:512], EXP)
                    nc.vector.tensor_scalar(
                        pt_i[:, 512:1024], ps[:, 512:1024], SCH_A, SCH_B,
                        mybir.AluOpType.mult, mybir.AluOpType.add)
                    return pt_i.bitcast(BF16)
                if on_dve:
                    pt_i = ptp.tile([128, 1024], I16, tag="pt")
                    nc.vector.tensor_scalar(
                        pt_i[:], ps[:], SCH_A, SCH_B,
                        mybir.AluOpType.mult, mybir.AluOpType.add)
                    return pt_i.bitcast(BF16)
                pt_t = ptp.tile([128, 1024], BF16, tag="pt")
                nc.scalar.activation(pt_t[:], ps[:], EXP)
                return pt_t

            def pv(pair, mt, pt_t, pso_a, pso_b):
                """Flipped PV: P^T chunk stationary, [V|1] moving."""
                for h2, pso in ((0, pso_a), (1, pso_b)):
                    head = 2 * pair + h2
                    for j in range(4):
                        nc.tensor.matmul(
                            pso[:, j, 0:65],
                            pt_t[:, h2 * 512 + j * 128:h2 * 512 + (j + 1) * 128],
                            v_sb[:, mt, head, :],
                            start=(mt == 0 and j == 0),
                            stop=(mt == MC - 1),
                            skip_group_check=True,
                        )

            def out_flush(pair, qc, pso_a, pso_b, on_dve):
                early = (pair == 0)
                for h2, pso in ((0, pso_a), (1, pso_b)):
                    o_sb = osb.tile([128, 4, 65], F32, tag="osb")
                    copy_ps(on_dve, o_sb[:], pso[:, :, 0:65])
                    ring = nc.gpsimd if early else nc.sync
                    ring.dma_start(out_ext[pair, qc, h2], o_sb[:])

            # ---- emission schedule: one flat stream of 256 units ----
            # Unit u = (pair, qc, mt): the S pair + exp for that m-tile.
            # Projection work rides as per-unit thunks just-in-time (late
            # projections double as PE padding in the exp-coupled phase);
            # PV matmuls drain from a FIFO backlog once their exp is
            # PV_LAG units old and (pair 0, qc<=1) the V tile exists.
            PV_LAG = 4
            units = [(p, q, m) for p in range(QC) for q in range(QC)
                     for m in range(MC)]
            uidx = {u: i for i, u in enumerate(units)}

            sched = {}

            def at(u, fn):
                sched.setdefault(u, []).append(fn)

            at(1, lambda dv: proj_k(0, 1, dv))
            at(3, lambda dv: proj_k(0, 2, dv))
            at(5, lambda dv: proj_k(0, 3, dv))
            v_unit = {m: 6 + 2 * m for m in range(MC)}
            for m in range(MC):
                at(v_unit[m], lambda dv, mm=m: proj_v(mm, dv))
            for p in range(QC):
                for q in range(QC):
                    if (p, q) == (0, 0):
                        continue
                    prev = uidx[(p, q, 0)] - (4 if (p, q) == (0, 1) else 6)
                    at(prev, lambda dv, pp=p, qq=q: proj_q(pp, qq, dv))
            for p in range(1, QC):
                # just-in-time K: consumed from unit 64p + 4mc
                for m in range(4):
                    at(64 * (p - 1) + 54 + 4 * m,
                       lambda dv, pp=p, mm=m: proj_k(pp, mm, dv))

            def v_ready(u, ent):
                p, q, mt = ent
                if p == 0 and q <= 1:
                    return u >= v_unit[mt] + 2
                return True

            backlog = []           # (unit_emitted, (pair, qc, mt), pt)
            cur = {"blk": None, "pso": None}

            def drain_one(u):
                eu, ent, pt_t = backlog[0]
                p, q, mt = ent
                if u is not None and (u < eu + PV_LAG or not v_ready(u, ent)):
                    return False
                backlog.pop(0)
                if cur["blk"] != (p, q):
                    cur["blk"] = (p, q)
                    pso_a = ps_op.tile([128, 4, 128], F32, tag="o")
                    pso_b = ps_op.tile([128, 4, 128], F32, tag="o")
                    cur["pso"] = (pso_a, pso_b)
                pv(p, mt, pt_t, *cur["pso"])
                if mt == MC - 1:
                    dv = (u is not None and
                          u % DVE_EVERY == DVE_EVERY // 2)
                    out_flush(p, q, *cur["pso"], dv)
                return True

            proj_k(0, 0)
            proj_q(0, 0)
            for u, (p, q, mt) in enumerate(units):
                # endgame (no proj padding left): strict alternation
                # halves the exp-latency chain seen by the PE stream
                if u >= 192:
                    on_dve = (u % 2) == 1
                else:
                    on_dve = (u % DVE_EVERY) == (DVE_EVERY // 2)
                pt_t = s_exp(p, q, mt, sched.get(u), on_dve, False)
                backlog.append((u, (p, q, mt), pt_t))
                budget = 3 if len(backlog) > 10 else (
                    2 if len(backlog) > 6 else 1)
                if u >= 240:
                    budget = 4
                for _ in range(budget):
                    if not backlog or not drain_one(u):
                        break
            while backlog:
                drain_one(None)

    nc.compile()
    return nc


def _get_nc(with_bias: bool):
    if with_bias not in _NC_CACHE:
        _NC_CACHE[with_bias] = _build(with_bias)
    return _NC_CACHE[with_bias]


def kernel(input, memory, Wq, bq, Wk, bk, Wv, bv):
    input = np.asarray(input, np.float32)
    memory = np.asarray(memory, np.float32)
    scale = HEAD_DIM ** -0.5
    with_bias = bool(np.any(bq) or np.any(bk) or np.any(bv))
    nc = _get_nc(with_bias)

    bf = ml_dtypes.bfloat16

    def prep_act(x):
        # [N, DIM] -> [4, 128, cc_n, 512]: qc-major, partition-major
        # contiguous per 512-col block (+ ones chunk for bias).
        xt = np.ascontiguousarray(x.T).reshape(CC, 128, QC, 512)
        if with_bias:
            aug = np.zeros((1, 128, QC, 512), np.float32)
            aug[0, 0] = 1.0
            xt = np.concatenate([xt, aug], axis=0)
        return np.ascontiguousarray(xt.transpose(2, 1, 0, 3).astype(bf))

    def prep_w(w, b, g, s=1.0):
        # [DIM, DIM] weight -> [4, 128, cc_n, 128] of (W.T * s): head-pair
        # major, partition-major contiguous.
        wt = (w.T[:, g * COLS:(g + 1) * COLS] * s).reshape(CC, 128, QC, 128)
        if with_bias:
            aug = np.zeros((1, 128, QC, 128), np.float32)
            aug[0, 0] = (np.asarray(b, np.float32)[g * COLS:(g + 1) * COLS]
                         * s).reshape(QC, 128)
            wt = np.concatenate([wt, aug], axis=0)
        return np.ascontiguousarray(wt.transpose(2, 1, 0, 3).astype(bf))

    def prep_wv(w, b, g):
        # -> [128, cc_n, 512] partition-major contiguous
        wt = np.asarray(w, np.float32).T[:, g * COLS:(g + 1) * COLS]
        wt = wt.reshape(CC, 128, COLS)
        if with_bias:
            aug = np.zeros((1, 128, COLS), np.float32)
            aug[0, 0] = np.asarray(b, np.float32)[g * COLS:(g + 1) * COLS]
            wt = np.concatenate([wt, aug], axis=0)
        return np.ascontiguousarray(wt.transpose(1, 0, 2).astype(bf))

    in_maps = []
    for c in range(N_CORES):
        b_idx, g = divmod(c, 2)
        in_maps.append({
            "xt": prep_act(input[b_idx]),
            "mt": prep_act(memory[b_idx]),
            "wq": prep_w(np.asarray(Wq, np.float32), bq, g, scale),
            "wk": prep_w(np.asarray(Wk, np.float32), bk, g),
            "wv": prep_wv(Wv, bv, g),
        })

    kw = dict(_RUN_KWARGS)
    res = run_bass_kernel_spmd(nc, in_maps, list(range(N_CORES)), **kw)
    global LAST_RESULT
    LAST_RESULT = res

    out = np.empty((B, N, DIM), np.float32)
    for c in range(N_CORES):
        b_idx, g = divmod(c, 2)
        o = res.results[c]["out"]            # [pair, qc, h2, 128, 4, 65]
        norm = o[..., :64] / o[..., 64:65]   # [pair, qc, h2, 128, 4, 64]
        # axes: (pair, qc, h2, qrow, j, d) -> q = qc*512 + j*128 + qrow,
        # col = (2*pair + h2)*64 + d
        norm = norm.transpose(1, 4, 3, 0, 2, 5)      # [qc, j, qrow, pair, h2, d]
        out[b_idx, :, g * COLS:(g + 1) * COLS] = norm.reshape(N, COLS)
    return out


# revision 25
# speedup vs baseline: 1.1965x; 1.1965x over previous
"""Multi-head cross-attention on 8 Trainium2 NeuronCores.

Problem (hardcoded): input [4, 2048, 1024], memory [4, 2048, 1024],
Wq/Wk/Wv [1024, 1024], bq/bk/bv [1024]; 16 heads x 64 dim; out
[4, 2048, 1024] f32.

Sharding: core c handles batch b = c//2 and head group g = c%2 (8
heads, output columns 512g:512g+512). Embarrassingly parallel - no
collectives.

Device dataflow (per core), everything contracted over channels with
host-pre-transposed operands so no on-chip transposes are needed:
  Q^T[d, n]  = sum_c WqT[c, d] * XT[c, n]     (depth scale folded in WqT)
  K^T[d, m]  = sum_c WkT[c, d] * MT[c, m]
  V[m, d]    = sum_c MT[c, m] * WvT[c, d]
  S^T[m, q]  = sum_d K^T[d, m] * Q^T[d, q]    (per head; heads of a pair
                                               row-packed in the PE array)
  P^T        = exp(S^T)                        (no max subtraction;
                                               logits are O(5), safe)
  O[q, d]    = sum_m P^T[m, q]^T [V | 1][m, d] ("flipped" PV: P^T chunk
               is the STATIONARY operand, [V|1] (65 cols) the moving one;
               65-cycle streams instead of 512 - half the PE cycles of
               the unflipped form. Ones column gives softmax sums. pso
               tiles are exactly one PSUM bank; only the first matmul
               into a tile carries start=True since start zeroes the
               whole bank.)
Host divides O[:, :64]/O[:, 64] (softmax normalization) and interleaves
head columns. Biases (zero here) are handled exactly via an extra K=1
contraction chunk when any bias is nonzero.

Engine budget: PE is the bottleneck (projections 82us + S 109us +
flipped PV 58us of stream + issue overhead). Exp of 33.5M logits costs
~1.15us per [128,1024] tile on ScalarE (292us > PE), so every 3rd tile
runs on DVE via a 1-pass Schraudolph exp in bf16 bit space:
i16 = round(S*128/ln2 + (127*128-7.42)), bitcast bf16 ~ exp(S) (1.8%
RMS on those tiles -> ~8.6e-3 total vs the 2e-2 gate). Input DMAs are
laid out host-side so every transfer is partition-major contiguous
(2-8KB packets; 128-col strided slices packetize at 256B and run at
~30GB/s vs ~190GB/s). Inputs ride sync HWDGE + a small scalar-ring
prefix (before exp #0 exists) + gpsimd SWDGE; outputs ride SWDGE.
"""

import numpy as np
import ml_dtypes

import concourse.bass as bass
import concourse.mybir as mybir
from concourse import bacc, tile
from concourse.bass_utils import run_bass_kernel_spmd

B, N, M, DIM = 4, 2048, 2048, 1024
NUM_HEADS, HEAD_DIM = 16, 64
HG = 8            # heads per core
COLS = HG * HEAD_DIM  # 512 output cols per core
N_CORES = 8
CC = DIM // 128   # 8 contraction chunks of 128
QC = 4            # q chunks of 512
MC = 16           # m (key) tiles of 128

F32 = mybir.dt.float32
BF16 = mybir.dt.bfloat16
I16 = mybir.dt.int16
EXP = mybir.ActivationFunctionType.Exp
CPY = mybir.ActivationFunctionType.Copy

# Schraudolph exp in bf16 bit space (round-to-nearest f32->i16 on DVE)
SCH_A = float(128.0 / np.log(2.0))
SCH_B = float(127 * 128) - 7.42
# units whose exp runs on DVE (every 3rd -> 1/3 of logits)
DVE_EVERY = 3

_NC_CACHE = {}
_RUN_KWARGS = {}   # test harness may inject trace=True etc.
LAST_RESULT = None


def _build(with_bias: bool):
    """Build the per-core SPMD Bass program."""
    cc_n = CC + (1 if with_bias else 0)
    nc = bacc.Bacc(None, target_bir_lowering=False)

    # All DRAM layouts are per-DMA-piece contiguous (partition-major):
    xt_ext = nc.declare_dram_parameter("xt", [QC, 128, cc_n, 512], BF16,
                                       isOutput=False)
    mt_ext = nc.declare_dram_parameter("mt", [QC, 128, cc_n, 512], BF16,
                                       isOutput=False)
    wq_ext = nc.declare_dram_parameter("wq", [QC, 128, cc_n, 128], BF16,
                                       isOutput=False)
    wk_ext = nc.declare_dram_parameter("wk", [QC, 128, cc_n, 128], BF16,
                                       isOutput=False)
    wv_ext = nc.declare_dram_parameter("wv", [128, cc_n, 512], BF16,
                                       isOutput=False)
    # out[pair, qc, h2] = [128 q rows, 4 q-subtiles, 64 dims + sum]
    out_ext = nc.declare_dram_parameter("out", [QC, QC, 2, 128, 4, 65], F32,
                                        isOutput=True)

    ch = [(i, 128) for i in range(CC)]
    if with_bias:
        ch.append((CC, 1))

    with tile.TileContext(nc) as tc:
        with (
            tc.tile_pool(name="acts", bufs=1) as acts,
            tc.tile_pool(name="qkv", bufs=1) as qkv,
            tc.tile_pool(name="pt", bufs=16) as ptp,
            tc.tile_pool(name="osb", bufs=6) as osb,
            tc.tile_pool(name="ps_proj", bufs=2, space="PSUM") as ps_proj,
            tc.tile_pool(name="ps_s", bufs=2, space="PSUM") as ps_sp,
            tc.tile_pool(name="ps_o", bufs=2, space="PSUM") as ps_op,
        ):
            wk_sb = acts.tile([128, QC, cc_n, 128], BF16)
            wq_sb = acts.tile([128, QC, cc_n, 128], BF16)
            xt_sb = acts.tile([128, QC, cc_n, 512], BF16)
            wv_sb = acts.tile([128, cc_n, 512], BF16)
            mt_sb = acts.tile([128, QC, cc_n, 512], BF16)

            # scalar-ring prefix: K0/Q00 stationaries + wv, all done by
            # ~12us (exp #0 can't issue earlier anyway); clear afterwards.
            nc.scalar.dma_start(wk_sb[:, 0], wk_ext[0])
            nc.scalar.dma_start(wq_sb[:, 0], wq_ext[0])
            nc.scalar.dma_start(wv_sb[:], wv_ext[:])
            # sync ring: mt chunks in K0-consumption order, then the bulk
            for c in range(QC):
                nc.sync.dma_start(mt_sb[:, c], mt_ext[c])
            for g in range(1, QC):
                nc.sync.dma_start(wk_sb[:, g], wk_ext[g])
            for g in range(1, QC):
                nc.sync.dma_start(wq_sb[:, g], wq_ext[g])
            nc.sync.dma_start(xt_sb[:, 2], xt_ext[2])
            nc.sync.dma_start(xt_sb[:, 3], xt_ext[3])
            # gpsimd SWDGE (aggregates packets, ~190GB/s): early xt blocks
            nc.gpsimd.dma_start(xt_sb[:, 0], xt_ext[0])
            nc.gpsimd.dma_start(xt_sb[:, 1], xt_ext[1])

            v_sb = qkv.tile([128, MC, HG, 65], BF16)   # V with ones col
            kt_sb = qkv.tile([128, QC, M], BF16)       # 2-head pairs stacked
            qt_sb = qkv.tile([128, QC, N], BF16)

            nc.gpsimd.memset(v_sb[:, :, :, 64:65], 1.0)

            def copy_ps(on_scalar, dst, src_ap):
                # PSUM->SBUF eviction on whichever exp engine is idle
                # this unit (Copy shares ScalarE's table set with Exp)
                if on_scalar:
                    nc.scalar.activation(dst, src_ap, CPY)
                else:
                    nc.vector.tensor_copy(dst, src_ap)

            def proj_k(pair, mc, on_dve=False):
                ps = ps_proj.tile([128, 512], F32, tag="proj")
                for j, (ci, rows) in enumerate(ch):
                    nc.tensor.matmul(
                        ps[:],
                        wk_sb[:rows, pair, ci, :],
                        mt_sb[:rows, mc, ci, :],
                        start=(j == 0), stop=(j == len(ch) - 1),
                    )
                copy_ps(on_dve, kt_sb[:, pair, mc * 512:(mc + 1) * 512], ps[:])

            def proj_q(pair, qc, on_dve=False):
                ps = ps_proj.tile([128, 512], F32, tag="proj")
                for j, (ci, rows) in enumerate(ch):
                    nc.tensor.matmul(
                        ps[:],
                        wq_sb[:rows, pair, ci, :],
                        xt_sb[:rows, qc, ci, :],
                        start=(j == 0), stop=(j == len(ch) - 1),
                    )
                copy_ps(on_dve, qt_sb[:, pair, qc * 512:(qc + 1) * 512], ps[:])

            def proj_v(mt, on_dve=False):
                ps = ps_proj.tile([128, 512], F32, tag="proj")
                for j, (ci, rows) in enumerate(ch):
                    nc.tensor.matmul(
                        ps[:],
                        mt_sb[:rows, mt // 4, ci,
                              (mt % 4) * 128:(mt % 4 + 1) * 128],
                        wv_sb[:rows, ci, :],
                        start=(j == 0), stop=(j == len(ch) - 1),
                    )
                copy_ps(on_dve, v_sb[:, mt, :, 0:64],
                        ps[:].rearrange("p (h d) -> p h d", h=HG))

            def s_exp(pair, qc, mt, extra, on_dve, split):
                """One m-tile: both heads' S matmuls into one PSUM tile,
                then one exp (ScalarE table exp, or DVE Schraudolph into
                int16-as-bf16 bit space). In split mode both engines
                each take one head's half concurrently - half the exp
                latency for the endgame where no proj padding remains."""
                ps = ps_sp.tile([128, 1024], F32, tag="s")
                for h2 in range(2):
                    d0 = 64 * h2
                    nc.tensor.matmul(
                        ps[:, h2 * 512:(h2 + 1) * 512],
                        kt_sb[d0:d0 + 64, pair, mt * 128:(mt + 1) * 128],
                        qt_sb[d0:d0 + 64, pair, qc * 512:(qc + 1) * 512],
                        start=True, stop=True,
                    )
                for fn in (extra or ()):
                    fn(on_dve)
                if split:
                    pt_i = ptp.tile([128, 1024], I16, tag="pt")
                    nc.scalar.activation(
                        pt_i.bitcast(BF16)[:, 0:512], ps[:, 0:512], EXP)
                    nc.vector.tensor_scalar(
                        pt_i[:, 512:1024], ps[:, 512:1024], SCH_A, SCH_B,
                        mybir.AluOpType.mult, mybir.AluOpType.add)
                    return pt_i.bitcast(BF16)
                if on_dve:
                    pt_i = ptp.tile([128, 1024], I16, tag="pt")
                    nc.vector.tensor_scalar(
                        pt_i[:], ps[:], SCH_A, SCH_B,
                        mybir.AluOpType.mult, mybir.AluOpType.add)
                    return pt_i.bitcast(BF16)
                pt_t = ptp.tile([128, 1024], BF16, tag="pt")
                nc.scalar.activation(pt_t[:], ps[:], EXP)
                return pt_t

            def pv(pair, mt, pt_t, pso_a, pso_b):
                """Flipped PV: P^T chunk stationary, [V|1] moving."""
                for h2, pso in ((0, pso_a), (1, pso_b)):
                    head = 2 * pair + h2
                    for j in range(4):
                        nc.tensor.matmul(
                            pso[:, j, 0:65],
                            pt_t[:, h2 * 512 + j * 128:h2 * 512 + (j + 1) * 128],
                            v_sb[:, mt, head, :],
                            start=(mt == 0 and j == 0),
                            stop=(mt == MC - 1),
                            skip_group_check=True,
                        )

            def out_flush(pair, qc, pso_a, pso_b, on_dve):
                early = (pair == 0)
                for h2, pso in ((0, pso_a), (1, pso_b)):
                    o_sb = osb.tile([128, 4, 65], F32, tag="osb")
                    copy_ps(on_dve, o_sb[:], pso[:, :, 0:65])
                    ring = nc.gpsimd if early else nc.sync
                    ring.dma_start(out_ext[pair, qc, h2], o_sb[:])

            # ---- emission schedule: one flat stream of 256 units ----
            # Unit u = (pair, qc, mt): the S pair + exp for that m-tile.
            # Projection work rides as per-unit thunks just-in-time (late
            # projections double as PE padding in the exp-coupled phase);
            # PV matmuls drain from a FIFO backlog once their exp is
            # PV_LAG units old and (pair 0, qc<=1) the V tile exists.
            PV_LAG = 4
            units = [(p, q, m) for p in range(QC) for q in range(QC)
                     for m in range(MC)]
            uidx = {u: i for i, u in enumerate(units)}

            sched = {}

            def at(u, fn):
                sched.setdefault(u, []).append(fn)

            at(1, lambda dv: proj_k(0, 1, dv))
            at(3, lambda dv: proj_k(0, 2, dv))
            at(5, lambda dv: proj_k(0, 3, dv))
            v_unit = {m: 6 + 2 * m for m in range(MC)}
            for m in range(MC):
                at(v_unit[m], lambda dv, mm=m: proj_v(mm, dv))
            for p in range(QC):
                for q in range(QC):
                    if (p, q) == (0, 0):
                        continue
                    prev = uidx[(p, q, 0)] - (4 if (p, q) == (0, 1) else 6)
                    at(prev, lambda dv, pp=p, qq=q: proj_q(pp, qq, dv))
            for p in range(1, QC):
                # just-in-time K: consumed from unit 64p + 4mc
                for m in range(4):
                    at(64 * (p - 1) + 54 + 4 * m,
                       lambda dv, pp=p, mm=m: proj_k(pp, mm, dv))

            def v_ready(u, ent):
                p, q, mt = ent
                if p == 0 and q <= 1:
                    return u >= v_unit[mt] + 2
                return True

            backlog = []           # (unit_emitted, (pair, qc, mt), pt)
            cur = {"blk": None, "pso": None}

            def drain_one(u):
                eu, ent, pt_t = backlog[0]
                p, q, mt = ent
                if u is not None and (u < eu + PV_LAG or not v_ready(u, ent)):
                    return False
                backlog.pop(0)
                if cur["blk"] != (p, q):
                    cur["blk"] = (p, q)
                    pso_a = ps_op.tile([128, 4, 128], F32, tag="o")
                    pso_b = ps_op.tile([128, 4, 128], F32, tag="o")
                    cur["pso"] = (pso_a, pso_b)
                pv(p, mt, pt_t, *cur["pso"])
                if mt == MC - 1:
                    dv = (u is not None and
                          u % DVE_EVERY == DVE_EVERY // 2)
                    out_flush(p, q, *cur["pso"], dv)
                return True

            proj_k(0, 0)
            proj_q(0, 0)
            for u, (p, q, mt) in enumerate(units):
                # NOTE: >1/3 DVE exp or split tiles regress - ScalarE exp
                # alone saturates PSUM read bw; concurrent readers split it
                on_dve = (u % DVE_EVERY) == (DVE_EVERY // 2)
                pt_t = s_exp(p, q, mt, sched.get(u), on_dve, False)
                backlog.append((u, (p, q, mt), pt_t))
                budget = 3 if len(backlog) > 10 else (
                    2 if len(backlog) > 6 else 1)
                if u >= 228:
                    budget = 4
                for _ in range(budget):
                    if not backlog or not drain_one(u):
                        break
            while backlog:
                drain_one(None)

    nc.compile()
    return nc


def _get_nc(with_bias: bool):
    if with_bias not in _NC_CACHE:
        _NC_CACHE[with_bias] = _build(with_bias)
    return _NC_CACHE[with_bias]


def kernel(input, memory, Wq, bq, Wk, bk, Wv, bv):
    input = np.asarray(input, np.float32)
    memory = np.asarray(memory, np.float32)
    scale = HEAD_DIM ** -0.5
    with_bias = bool(np.any(bq) or np.any(bk) or np.any(bv))
    nc = _get_nc(with_bias)

    bf = ml_dtypes.bfloat16

    def prep_act(x):
        # [N, DIM] -> [4, 128, cc_n, 512]: qc-major, partition-major
        # contiguous per 512-col block (+ ones chunk for bias).
        xt = np.ascontiguousarray(x.T).reshape(CC, 128, QC, 512)
        if with_bias:
            aug = np.zeros((1, 128, QC, 512), np.float32)
            aug[0, 0] = 1.0
            xt = np.concatenate([xt, aug], axis=0)
        return np.ascontiguousarray(xt.transpose(2, 1, 0, 3).astype(bf))

    def prep_w(w, b, g, s=1.0):
        # [DIM, DIM] weight -> [4, 128, cc_n, 128] of (W.T * s): head-pair
        # major, partition-major contiguous.
        wt = (w.T[:, g * COLS:(g + 1) * COLS] * s).reshape(CC, 128, QC, 128)
        if with_bias:
            aug = np.zeros((1, 128, QC, 128), np.float32)
            aug[0, 0] = (np.asarray(b, np.float32)[g * COLS:(g + 1) * COLS]
                         * s).reshape(QC, 128)
            wt = np.concatenate([wt, aug], axis=0)
        return np.ascontiguousarray(wt.transpose(2, 1, 0, 3).astype(bf))

    def prep_wv(w, b, g):
        # -> [128, cc_n, 512] partition-major contiguous
        wt = np.asarray(w, np.float32).T[:, g * COLS:(g + 1) * COLS]
        wt = wt.reshape(CC, 128, COLS)
        if with_bias:
            aug = np.zeros((1, 128, COLS), np.float32)
            aug[0, 0] = np.asarray(b, np.float32)[g * COLS:(g + 1) * COLS]
            wt = np.concatenate([wt, aug], axis=0)
        return np.ascontiguousarray(wt.transpose(1, 0, 2).astype(bf))

    in_maps = []
    for c in range(N_CORES):
        b_idx, g = divmod(c, 2)
        in_maps.append({
            "xt": prep_act(input[b_idx]),
            "mt": prep_act(memory[b_idx]),
            "wq": prep_w(np.asarray(Wq, np.float32), bq, g, scale),
            "wk": prep_w(np.asarray(Wk, np.float32), bk, g),
            "wv": prep_wv(Wv, bv, g),
        })

    kw = dict(_RUN_KWARGS)
    res = run_bass_kernel_spmd(nc, in_maps, list(range(N_CORES)), **kw)
    global LAST_RESULT
    LAST_RESULT = res

    out = np.empty((B, N, DIM), np.float32)
    for c in range(N_CORES):
        b_idx, g = divmod(c, 2)
        o = res.results[c]["out"]            # [pair, qc, h2, 128, 4, 65]
        norm = o[..., :64] / o[..., 64:65]   # [pair, qc, h2, 128, 4, 64]
        # axes: (pair, qc, h2, qrow, j, d) -> q = qc*512 + j*128 + qrow,
        # col = (2*pair + h2)*64 + d
        norm = norm.transpose(1, 4, 3, 0, 2, 5)      # [qc, j, qrow, pair, h2, d]
        out[b_idx, :, g * COLS:(g + 1) * COLS] = norm.reshape(N, COLS)
    return out
